# revision 1
# baseline (speedup 1.0000x reference)
"""Trainium2 Bass kernel for a dense transformer encoder layer.

Problem shapes: B=4, S=1024, D=1024, H=16, DK=64, DFF=4096 (f32 I/O).
Returns (out [B,S,D], attn_w_last_head [B,S,S]) like the reference.

Sharding: 8 cores = 4 batches x 2 sequence halves. Each core computes
512 query rows of one batch end-to-end (K/V projections for the full
sequence are duplicated within each pair) -- no collectives.

All matmul operands are bf16 (fp32 PSUM accumulation); residual adds,
layernorms and outputs stay fp32.  Dataflow on each core:

  V[s,hdk]   = x_vT.T @ wv   (+ones col per head -> softmax denominator)
  QT[h],KT[h] per 128-feature M-tile (2 heads), padded to K=128
  per head (software-pipelined on PE, exp on ACT):
    sT[h]  = KT[h].T @ QT[h]           [1024, 512]  scores transposed
    eT[h]  = exp(sT + mask)            bf16 (ACT, 2 key-tiles per op)
    aT[h]  = (V'[h]).T @ eT[h]         [65, 512]  (row 64 = denom)
    concatT[h cols] = aT[0:64] * bcast(recip(denom))  (GpSimd bcast)
  mha[q,:]  = concatT.T @ w0 (+b0) + x_q -> LN1 -> sub1
  attn_w15  = exp(Q15.T K15) * recip(rowsum)  (recomputed q-major, f32)
  sub1T (PE transpose, bf16) ; hT[f] = relu(ff1_w.T @ sub1T)  [4096,512]
  ffn[q,:]  = sum_f hT[f].T @ ff2_w[f,:] (+ff2_b) ; +sub1 -> LN2 -> out

Weights/activations are pre-tiled AND pre-cast to bf16 on the host so
every SBUF tile loads as one DMA of contiguous per-partition rows.
"""

import numpy as np

B, S, D, H, DK, DFF = 4, 1024, 1024, 16, 64, 4096
EPS = 1e-6
P = 128
SQ = 512          # query rows per core
NQT = SQ // P     # 4 query-row tiles
NKT = D // P      # 8 contraction tiles over D
NST = S // P      # 8 key tiles
NFT = DFF // P    # 32 dff tiles


def _layernorm(nc, pool, x, eps_t, g_sb, b_sb, out_dma=None):
    """In-place layernorm over the free dim (D=1024) of x [128, 1024].

    With out_dma set (and no affine), the final normalize is split into
    halves with the output DMA issued per half to shorten the tail."""
    from concourse import mybir
    AF = mybir.ActivationFunctionType
    ALU = mybir.AluOpType
    F32 = mybir.dt.float32
    xr = x.rearrange("p (a b) -> p a b", b=512)
    stats = pool.tile([P, 2, 6], F32, tag="lnstats", name="lnstats")
    for i in range(2):
        nc.vector.bn_stats(stats[:, i, :], xr[:, i, :])
    mv = pool.tile([P, 2], F32, tag="lnmv", name="lnmv")
    nc.vector.bn_aggr(mv, stats)
    rstd = pool.tile([P, 1], F32, tag="lnrstd", name="lnrstd")
    nc.scalar.activation(rstd, mv[:, 1:2], AF.Sqrt, bias=eps_t)
    nc.vector.reciprocal(rstd, rstd)
    if out_dma is not None and g_sb is None and b_sb is None:
        for hh in range(2):
            sl = slice(hh * 512, hh * 512 + 512)
            nc.vector.tensor_scalar(out=x[:, sl], in0=x[:, sl],
                                    scalar1=mv[:, 0:1], scalar2=rstd,
                                    op0=ALU.subtract, op1=ALU.mult)
            nc.sync.dma_start(out_dma[:, sl], x[:, sl])
        return
    nc.vector.tensor_scalar(out=x, in0=x, scalar1=mv[:, 0:1], scalar2=rstd,
                            op0=ALU.subtract, op1=ALU.mult)
    if g_sb is not None:
        nc.vector.tensor_mul(x, x, g_sb)
    if b_sb is not None:
        nc.vector.tensor_add(x, x, b_sb)
    if out_dma is not None:
        nc.sync.dma_start(out_dma, x)


def _build(flags):
    import concourse.bass as bass
    import concourse.tile as tile
    from concourse import bacc, mybir
    from concourse.masks import make_identity
    from contextlib import ExitStack

    dt = mybir.dt
    AF = mybir.ActivationFunctionType
    ALU = mybir.AluOpType
    F32, BF16 = dt.float32, dt.bfloat16

    nc = bacc.Bacc("TRN2", target_bir_lowering=False, debug=False)

    def din(name, shape, dtp=BF16):
        return nc.dram_tensor(name, shape, dtp, kind="ExternalInput").ap()

    def dout(name, shape):
        return nc.dram_tensor(name, shape, F32, kind="ExternalOutput").ap()

    # pre-tiled inputs (see make_in_maps for layouts)
    xq_t = din("xq_t", (P, NKT * SQ))
    xk_t = din("xk_t", (P, NKT * S))
    xv_t = din("xv_t", (NST, P, NKT * P))
    x_q_r = din("x_q_r", (SQ, D), F32)
    maskT = din("maskT", (S, 1), F32)
    wq_t = din("wq_t", (NKT, P, NKT * P))
    wk_t = din("wk_t", (NKT, P, NKT * P))
    wv_t = din("wv_t", (2, P, NKT * 512))
    bq2T = din("bq2T", (H * DK, 1), F32)
    bk2T = din("bk2T", (H * DK, 1), F32)
    bv2 = din("bv2", (1, H * DK))
    w0_t = din("w0_t", (P, NKT * D))
    b0 = din("b0", (1, D))
    f1_t = din("f1_t", (NFT, P, NKT * P))
    ff1_bT = din("ff1_bT", (DFF, 1), F32)
    f2_t = din("f2_t", (P, NFT * D))
    ff2_b = din("ff2_b", (1, D))
    ln1_g = din("ln1_g", (1, D), F32)
    ln1_b = din("ln1_b", (1, D), F32)
    ln2_g = din("ln2_g", (1, D), F32)
    ln2_b = din("ln2_b", (1, D), F32)

    out_r = dout("out_r", (SQ, D))
    attnw15 = dout("attnw15", (SQ, S))

    def bcast_row(ap_1xn):
        return bass.AP(tensor=ap_1xn.tensor, offset=ap_1xn.offset,
                       ap=[[0, P]] + list(ap_1xn.ap[1:]))

    def flat(t3):
        return t3.rearrange("p a b -> p (a b)")

    with tile.TileContext(nc) as tc, ExitStack() as top:
        consts = top.enter_context(tc.tile_pool(name="consts", bufs=1))
        persist = top.enter_context(tc.tile_pool(name="persist", bufs=1))

        ident = consts.tile([P, P], F32)
        eps_t = consts.tile([P, 1], F32)
        ones_row = None
        if flags["b0"] or flags["bv"] or flags["mask"] or flags["ff2b"]:
            ones_row = consts.tile([1, P], BF16)
            nc.gpsimd.memset(ones_row, 1.0)

        ln1g_sb = ln1b_sb = ln2g_sb = ln2b_sb = None
        if flags["mask"]:
            mrow = consts.tile([P, NST], F32)
            nc.sync.dma_start(mrow, maskT.rearrange("(t p) o -> p (t o)", p=P))
            nc.scalar.mul(mrow, mrow, -1e9)
            # bf16 masked row for the q-major attnw15 recompute
            mrow_f = consts.tile([1, S], F32)
            nc.sync.dma_start(mrow_f, maskT.rearrange("s o -> o s"))
            nc.scalar.mul(mrow_f, mrow_f, -1e9)
            mrow_r = consts.tile([1, S], BF16)
            nc.vector.tensor_copy(mrow_r, mrow_f)
        if flags["b0"]:
            b0_sb = consts.tile([1, D], BF16)
            nc.sync.dma_start(b0_sb, b0)
        if flags["bv"]:
            bv2_sb = consts.tile([1, H * DK], BF16)
            nc.sync.dma_start(bv2_sb, bv2)
        if flags["bq"]:
            bqT_sb = consts.tile([P, NST], F32)
            nc.sync.dma_start(bqT_sb, bq2T.rearrange("(t p) o -> p (t o)", p=P))
        if flags["bk"]:
            bkT_sb = consts.tile([P, NST], F32)
            nc.sync.dma_start(bkT_sb, bk2T.rearrange("(t p) o -> p (t o)", p=P))
        if flags["ff1b"]:
            f1bT_sb = consts.tile([P, NFT], F32)
            nc.sync.dma_start(f1bT_sb, ff1_bT.rearrange("(t p) o -> p (t o)", p=P))
        if flags["ff2b"]:
            f2b_sb = consts.tile([1, D], BF16)
            nc.sync.dma_start(f2b_sb, ff2_b)
        if flags["ln1g"]:
            ln1g_sb = consts.tile([P, D], F32, tag="ln1g")
            nc.sync.dma_start(ln1g_sb, bcast_row(ln1_g))
        if flags["ln1b"]:
            ln1b_sb = consts.tile([P, D], F32, tag="ln1b")
            nc.sync.dma_start(ln1b_sb, bcast_row(ln1_b))
        if flags["ln2g"]:
            ln2g_sb = consts.tile([P, D], F32, tag="ln2g")
            nc.sync.dma_start(ln2g_sb, bcast_row(ln2_g))
        if flags["ln2b"]:
            ln2b_sb = consts.tile([P, D], F32, tag="ln2b")
            nc.sync.dma_start(ln2b_sb, bcast_row(ln2_b))

        # transposed, normalized attention output [cin, q] (bf16)
        concatT = persist.tile([P, NKT, SQ], BF16, tag="concatT", name="concatT")
        # w0 lives in the top scope so its DMA can issue with the attention
        # loads; w1 streams with a prefetch ring that starts during W0
        w0_sb = persist.tile([P, NKT, D], BF16, tag="w0t")
        w1pool = top.enter_context(tc.tile_pool(name="w1p", bufs=6))
        w1_tiles = {}

        def w1_fetch(ft):
            w1 = w1pool.tile([P, NKT, P], BF16, tag="w1t", name=f"w1_{ft}")
            nc.sync.dma_start(flat(w1), f1_t[ft])
            w1_tiles[ft] = w1

        # ---------------- attention: single pass, 16 heads ----------------
        attn_scope = top.enter_context(ExitStack())
        hpool = attn_scope.enter_context(tc.tile_pool(name="hs", bufs=1))
        # q/k per-head tiles, zero-padded to K=128 partitions
        qt_pad = hpool.tile([P, H, SQ], BF16, tag="qt")
        kt_pad = hpool.tile([P, H, S], BF16, tag="kt")
        v_sb = hpool.tile([P, NST, H, DK + 1], BF16, tag="v")
        recip15 = hpool.tile([P, NQT], F32, tag="recip15")

        with ExitStack() as ph:
            xpool = ph.enter_context(tc.tile_pool(name="xs", bufs=1))
            vscope = ExitStack()
            vxpool = vscope.enter_context(tc.tile_pool(name="vx", bufs=1))

            # kick off all attention DMAs before any on-chip init so the
            # first V matmul can start as early as possible
            wv_sb = vxpool.tile([P, 2, NKT, 512], BF16, tag="wv")
            wvf = wv_sb.rearrange("p h a b -> p (h a b)")
            nc.sync.dma_start(wvf[:, 0:4 * 512], wv_t[0][:, 0:4 * 512])
            nc.sync.dma_start(wvf[:, 4 * 512:NKT * 512], wv_t[0][:, 4 * 512:])
            xv_ts = []
            for km in range(NST):
                xv = vxpool.tile([P, NKT, P], BF16, tag=f"xv{km}",
                                 name=f"xv{km}")
                nc.sync.dma_start(flat(xv), xv_t[km])
                xv_ts.append(xv)
            nc.sync.dma_start(wv_sb[:, 1].rearrange("p a b -> p (a b)"), wv_t[1])
            xq_sb = xpool.tile([P, NKT, SQ], BF16, tag="xq")
            nc.sync.dma_start(flat(xq_sb), xq_t)
            xk_sb = xpool.tile([P, NKT, S], BF16, tag="xk")
            nc.sync.dma_start(flat(xk_sb)[:, 0:NKT * 512], xk_t[:, 0:NKT * 512])
            nc.sync.dma_start(flat(xk_sb)[:, NKT * 512:], xk_t[:, NKT * 512:])
            nc.sync.dma_start(flat(w0_sb), w0_t)

            # on-chip constant init on the (otherwise idle) GpSimd engine
            make_identity(nc, ident)
            nc.gpsimd.memset(eps_t, EPS)
            nc.gpsimd.memset(qt_pad[64:128, :, :], 0.0)
            nc.gpsimd.memset(kt_pad[64:128, :, :], 0.0)
            nc.vector.memset(v_sb[:, :, :, DK:DK + 1], 1.0)

            pp = ph.enter_context(tc.tile_pool(name="pp", bufs=2, space="PSUM"))
            ps = ph.enter_context(tc.tile_pool(name="ps", bufs=2, space="PSUM"))
            pa = ph.enter_context(tc.tile_pool(name="pa", bufs=2, space="PSUM"))

            # V projection first: all 16 heads, hdk-half outer so compute can
            # begin as soon as the first wv half lands
            for hf in range(2):
                for km in range(NST):
                    pv = pp.tile([P, 512], F32, tag="pp", name="pv")
                    for kt in range(NKT):
                        nc.tensor.matmul(pv, xv_ts[km][:, kt, :],
                                         wv_sb[:, hf, kt, :],
                                         start=(kt == 0),
                                         stop=(kt == NKT - 1 and not flags["bv"]))
                    if flags["bv"]:
                        nc.tensor.matmul(pv, ones_row,
                                         bv2_sb[:, hf * 512:hf * 512 + 512],
                                         start=False, stop=True)
                    nc.vector.tensor_copy(
                        v_sb[:, km, hf * 8:hf * 8 + 8, 0:DK],
                        pv.rearrange("p (s c) -> p s c", c=DK))
            # wv / xv space is dead from here on -- release it for the
            # eT / recip pools
            vscope.close()
            wpool = ph.enter_context(tc.tile_pool(name="w", bufs=3))
            epool = ph.enter_context(tc.tile_pool(name="e", bufs=3))
            rpool = ph.enter_context(tc.tile_pool(name="r", bufs=2))

            def scores_head(h):
                """8 score matmuls + exp into a fresh eT tile; returns eT."""
                eT = epool.tile([P, NST, SQ], BF16, tag="eT", name=f"eT{h}")
                for g in range(4):          # pairs of key tiles
                    psc = ps.tile([P, 2, 512], F32, tag="psc", name="psc")
                    for j in range(2):
                        st = 2 * g + j
                        nc.tensor.matmul(
                            psc[:, j, :],
                            kt_pad[:, h, st * P:(st + 1) * P],
                            qt_pad[:, h, :], start=True, stop=True)
                    if flags["mask"]:
                        for j in range(2):
                            st = 2 * g + j
                            nc.scalar.activation(
                                eT[:, st, :], psc[:, j, :], AF.Exp,
                                bias=mrow[:, st:st + 1])
                    else:
                        nc.scalar.activation(
                            eT[:, 2 * g:2 * g + 2, :], psc, AF.Exp)
                return eT

            den4s = {}

            def attnv_mm(h, eT):
                """aT = V'.T @ eT (row 64 = softmax denominator), evicted
                immediately: unnormalized values into concatT, denominator
                into the group's den4 row."""
                pat = pa.tile([DK + 1, SQ], F32, tag="pat", name="pat")
                for st in range(NST):
                    nc.tensor.matmul(pat, v_sb[:, st, h, :], eT[:, st, :],
                                     start=(st == 0), stop=(st == NST - 1))
                po = (h % 2) * 64
                nc.vector.tensor_copy(concatT[po:po + 64, h // 2, :],
                                      pat[0:DK, :])
                if h % 4 == 0:
                    # denominator rows parked at partitions 0/32/64/96 (SBUF
                    # writes must start 32-aligned); one batched reciprocal
                    # covers the whole group
                    den4s[h // 4] = rpool.tile([97, SQ], F32, tag="den4",
                                               name=f"den4_{h // 4}", bufs=3)
                j = (h % 4) * 32
                nc.vector.tensor_copy(den4s[h // 4][j:j + 1, :],
                                      pat[DK:DK + 1, :])

            rbs = {}

            def attnv_norm_recip(grp):
                """One batched reciprocal per 4 heads (amortizes the slow DVE
                reciprocal), then kick the GpSimd partition broadcasts.
                De-prioritized: the Tile scheduler treats this chain as
                filler so it never delays the K/psum eviction copies the PE
                is waiting on."""
                den4 = den4s.pop(grp)
                with tc.high_priority(offset=-250):
                    r4 = rpool.tile([97, SQ], F32, tag="r4", name=f"r4_{grp}",
                                    bufs=2)
                    nc.vector.reciprocal(r4, den4)
                    for j in range(4):
                        h = grp * 4 + j
                        rj = rpool.tile([1, SQ], BF16, tag="rj", name=f"rj{h}",
                                        bufs=4)
                        nc.vector.tensor_copy(rj, r4[j * 32:j * 32 + 1, :])
                        # 128-partition broadcast so the in-place multiply's
                        # SBUF operands share a start partition
                        rb = rpool.tile([P, SQ], BF16, tag="rb", name=f"rb{h}",
                                        bufs=4)
                        nc.gpsimd.partition_broadcast(rb, rj, channels=P)
                        rbs[h] = rb

            def attnv_norm_mult(grp):
                """In-place scale of concatT, de-prioritized like the recip
                chain (its only consumer is W0, far in the future)."""
                with tc.high_priority(offset=-250):
                    for j in range(4):
                        h = grp * 4 + j
                        rb = rbs.pop(h)
                        po = (h % 2) * 64
                        sl = concatT[po:po + 64, h // 2, :]
                        nc.vector.tensor_mul(sl, sl, rb[po:po + 64, :])

            # software pipeline: Q/K proj of tile mt, scores(h), attnV
            # matmuls of h-2.  Batched normalization groups are emitted at
            # M-tile boundaries so their DVE chain queues BEHIND the K
            # evictions the next projection's PSUM ring is waiting on.
            pending = []
            for mt in range(NKT):
                if mt in (3, 5, 7):
                    attnv_norm_recip((mt - 3) // 2)
                if mt in (4, 6):
                    attnv_norm_mult((mt - 4) // 2)
                wq_mt = wpool.tile([P, NKT, P], BF16, tag="wq_mt", name="wq_mt")
                nc.sync.dma_start(flat(wq_mt), wq_t[mt])
                pq = pp.tile([P, 512], F32, tag="pp", name="pq")
                for kt in range(NKT):
                    nc.tensor.matmul(pq, wq_mt[:, kt, :], xq_sb[:, kt, :],
                                     start=(kt == 0), stop=(kt == NKT - 1))
                for sub in range(2):
                    psl = slice(sub * 64, sub * 64 + 64)
                    # Q evictions ride the scalar engine to keep the DVE
                    # FIFO clear for the K / attnV eviction chain
                    nc.scalar.activation(
                        qt_pad[0:64, 2 * mt + sub, :], pq[psl, :],
                        AF.Identity,
                        bias=bqT_sb[psl, mt:mt + 1] if flags["bq"] else 0.0)

                wk_mt = wpool.tile([P, NKT, P], BF16, tag="wk_mt", name="wk_mt")
                nc.sync.dma_start(flat(wk_mt), wk_t[mt])
                for half in range(2):
                    fsl = slice(half * 512, half * 512 + 512)
                    pk = pp.tile([P, 512], F32, tag="pp", name="pk")
                    for kt in range(NKT):
                        nc.tensor.matmul(pk, wk_mt[:, kt, :],
                                         xk_sb[:, kt, fsl],
                                         start=(kt == 0), stop=(kt == NKT - 1))
                    for sub in range(2):
                        psl = slice(sub * 64, sub * 64 + 64)
                        if flags["bk"]:
                            nc.scalar.activation(
                                kt_pad[0:64, 2 * mt + sub, fsl], pk[psl, :],
                                AF.Identity, bias=bkT_sb[psl, mt:mt + 1])
                        else:
                            nc.vector.tensor_copy(
                                kt_pad[0:64, 2 * mt + sub, fsl], pk[psl, :])

                for h in (2 * mt, 2 * mt + 1):
                    eT = scores_head(h)
                    if len(pending) == 2:
                        hh, eTT = pending.pop(0)
                        attnv_mm(hh, eTT)
                    pending.append((h, eT))
            attnv_norm_mult(2)
            for hh, eTT in pending:
                attnv_mm(hh, eTT)
            attnv_norm_recip(3)
            attnv_norm_mult(3)

        # ---------------- mha out + residual + LN1 + attnw15 --------------
        sub1 = [persist.tile([P, D], F32, tag=f"sub1_{qt}", name=f"sub1_{qt}")
                for qt in range(NQT)]
        sub1T = persist.tile([P, NKT, SQ], BF16, tag="sub1T")
        with ExitStack() as ph:
            xpool = ph.enter_context(tc.tile_pool(name="xr", bufs=2))
            apool = ph.enter_context(tc.tile_pool(name="a15", bufs=2))
            lnpool = ph.enter_context(tc.tile_pool(name="ln1pool", bufs=4))
            po = ph.enter_context(tc.tile_pool(name="po", bufs=2, space="PSUM"))
            p15 = ph.enter_context(tc.tile_pool(name="p15", bufs=2, space="PSUM"))
            pt = ph.enter_context(tc.tile_pool(name="ptr", bufs=2, space="PSUM"))

            # start the FF1 weight stream while W0 runs
            for ft in range(6):
                w1_fetch(ft)

            def w0_block(qt):
                pmo = po.tile([P, 2, 512], F32, tag="pmo", name="pmo")
                for half in range(2):
                    fsl = slice(half * 512, half * 512 + 512)
                    for kt in range(NKT):
                        nc.tensor.matmul(pmo[:, half, :],
                                         concatT[:, kt, qt * P:(qt + 1) * P],
                                         w0_sb[:, kt, fsl],
                                         start=(kt == 0),
                                         stop=(kt == NKT - 1 and not flags["b0"]))
                    if flags["b0"]:
                        nc.tensor.matmul(pmo[:, half, :], ones_row,
                                         b0_sb[:, fsl], start=False, stop=True)
                xq = xpool.tile([P, D], F32, tag="xqr")
                nc.sync.dma_start(xq, x_q_r[qt * P:(qt + 1) * P, :])
                nc.vector.tensor_add(sub1[qt], flat(pmo), xq)
                _layernorm(nc, lnpool, sub1[qt], eps_t, ln1g_sb, ln1b_sb)

            def t_block(qt):
                # transpose sub1 (f32) into sub1T [c, q] (bf16 on evict)
                for ct in range(NKT):
                    ptt = pt.tile([P, P], F32, tag="ptt", name="ptt")
                    nc.tensor.transpose(
                        ptt, sub1[qt][:, ct * P:(ct + 1) * P], ident)
                    nc.vector.tensor_copy(
                        sub1T[:, ct, qt * P:(qt + 1) * P], ptt)

            # interleave so each transpose block's LN1 is ready when the PE
            # reaches it
            w0_block(0)
            for qt in range(1, NQT):
                w0_block(qt)
                t_block(qt - 1)
            t_block(NQT - 1)

            # head-15 attention weights: recompute scores q-major (f32 path).
            # De-prioritized: pure filler for gaps during the FF1 ramp so its
            # ACT/DVE work never delays LN1 or the sub1T transposes.
            with tc.high_priority(offset=-180):
                for qt in range(NQT):
                    a15 = apool.tile([P, S], F32, tag="a15")
                    for half in range(2):
                        fsl = slice(half * 512, half * 512 + 512)
                        pw = p15.tile([P, 512], F32, tag="p15", name="pw")
                        nc.tensor.matmul(
                            pw, qt_pad[:, H - 1, qt * P:(qt + 1) * P],
                            kt_pad[:, H - 1, fsl],
                            start=True, stop=not flags["mask"])
                        if flags["mask"]:
                            nc.tensor.matmul(pw, ones_row, mrow_r[:, fsl],
                                             start=False, stop=True)
                        nc.scalar.activation(a15[:, fsl], pw, AF.Exp)
                    den = lnpool.tile([P, 1], F32, tag="lnrstd", name="den15")
                    nc.vector.tensor_reduce(den, a15, mybir.AxisListType.X,
                                            ALU.add)
                    rc = recip15[:, qt:qt + 1]
                    nc.vector.reciprocal(rc, den)
                    nc.vector.tensor_scalar_mul(out=a15, in0=a15, scalar1=rc)
                    nc.sync.dma_start(attnw15[qt * P:(qt + 1) * P, :], a15)

        attn_scope.close()

        # ---------------- FFN + residual + LN2 ----------------
        with ExitStack() as ph:
            mpool = ph.enter_context(tc.tile_pool(name="f", bufs=1))
            w2pool = ph.enter_context(tc.tile_pool(name="w2p", bufs=1))
            lnpool = ph.enter_context(tc.tile_pool(name="ln2pool", bufs=4))
            pf = ph.enter_context(tc.tile_pool(name="pf", bufs=3, space="PSUM"))
            pg = ph.enter_context(tc.tile_pool(name="pg", bufs=2, space="PSUM"))

            w2_sb = w2pool.tile([P, NFT, D], BF16, tag="w2t")

            hT = mpool.tile([P, NFT, SQ], BF16, tag="hT")
            for ft in range(NFT):
                if ft + 6 < NFT:
                    w1_fetch(ft + 6)
                if ft == 16:
                    # FF2 weights: issued midway through FF1 so they queue
                    # behind the remaining FF1 stream but land before FF2
                    for c in range(8):
                        nc.sync.dma_start(
                            w2_sb[:, 4 * c:4 * c + 4, :],
                            f2_t.rearrange("p (a b) -> p a b", b=D)
                            [:, 4 * c:4 * c + 4, :])
                w1 = w1_tiles.pop(ft)
                pff = pf.tile([P, SQ], F32, tag="pff", name="pff")
                for kt in range(NKT):
                    nc.tensor.matmul(pff, w1[:, kt, :], sub1T[:, kt, :],
                                     start=(kt == 0), stop=(kt == NKT - 1))
                if flags["ff1b"]:
                    nc.vector.tensor_scalar(
                        out=hT[:, ft, :], in0=pff,
                        scalar1=f1bT_sb[:, ft:ft + 1], scalar2=0.0,
                        op0=ALU.add, op1=ALU.max)
                else:
                    nc.vector.tensor_scalar_max(
                        out=hT[:, ft, :], in0=pff, scalar1=0.0)

            for qt in range(NQT):
                pfn = pg.tile([P, 2, 512], F32, tag="pfn", name="pfn")
                for half in range(2):
                    fsl = slice(half * 512, half * 512 + 512)
                    for ft in range(NFT):
                        nc.tensor.matmul(
                            pfn[:, half, :],
                            hT[:, ft, qt * P:(qt + 1) * P],
                            w2_sb[:, ft, fsl],
                            start=(ft == 0),
                            stop=(ft == NFT - 1 and not flags["ff2b"]))
                    if flags["ff2b"]:
                        nc.tensor.matmul(pfn[:, half, :], ones_row,
                                         f2b_sb[:, fsl], start=False, stop=True)
                nc.vector.tensor_add(sub1[qt], flat(pfn), sub1[qt])
                _layernorm(nc, lnpool, sub1[qt], eps_t, ln2g_sb, ln2b_sb,
                           out_dma=out_r[qt * P:(qt + 1) * P, :])

    nc.compile()
    return nc


_CACHE = {}


def _get_program(flags):
    key = tuple(sorted(flags.items()))
    if key not in _CACHE:
        _CACHE[key] = _build(flags)
    return _CACHE[key]


def make_flags(mask, bq, bk, bv, b0, ff1_b, ff2_b, ln1_g, ln1_b, ln2_g, ln2_b):
    return {
        "mask": bool(np.any(mask)), "bq": bool(np.any(bq)),
        "bk": bool(np.any(bk)), "bv": bool(np.any(bv)),
        "b0": bool(np.any(b0)), "ff1b": bool(np.any(ff1_b)),
        "ff2b": bool(np.any(ff2_b)),
        "ln1g": bool(np.any(ln1_g != 1.0)), "ln1b": bool(np.any(ln1_b)),
        "ln2g": bool(np.any(ln2_g != 1.0)), "ln2b": bool(np.any(ln2_b)),
    }


def make_in_maps(x_v, x_k, x_q, mask, wq, bq, wk, bk, wv, bv, w0, b0,
                 ln1_g, ln1_b, ff1_w, ff1_b, ff2_w, ff2_b, ln2_g, ln2_b):
    import ml_dtypes
    f32 = np.float32
    bf16 = ml_dtypes.bfloat16
    c = np.ascontiguousarray

    def cb(a):
        return c(np.asarray(a, f32).astype(bf16))

    wq2 = np.transpose(np.asarray(wq, f32), (1, 0, 2)).reshape(D, H * DK) / 8.0
    wk2 = np.transpose(np.asarray(wk, f32), (1, 0, 2)).reshape(D, H * DK)
    wv2 = np.transpose(np.asarray(wv, f32), (1, 0, 2)).reshape(D, H * DK)
    w0a = np.asarray(w0, f32)
    f1a = np.asarray(ff1_w, f32)
    f2a = np.asarray(ff2_w, f32)
    shared = {
        # [mt, p, kt*128+c] = wq2[kt*128+p, mt*128+c]
        "wq_t": cb(wq2.reshape(NKT, P, NKT, P).transpose(2, 1, 0, 3)
                   .reshape(NKT, P, NKT * P)),
        "wk_t": cb(wk2.reshape(NKT, P, NKT, P).transpose(2, 1, 0, 3)
                   .reshape(NKT, P, NKT * P)),
        # [hf, p, kt*512+c] = wv2[kt*128+p, hf*512+c]
        "wv_t": cb(wv2.reshape(NKT, P, 2, 512).transpose(2, 1, 0, 3)
                   .reshape(2, P, NKT * 512)),
        "bq2T": c(np.asarray(bq, f32).reshape(H * DK, 1) / 8.0),
        "bk2T": c(np.asarray(bk, f32).reshape(H * DK, 1)),
        "bv2": cb(np.asarray(bv, f32).reshape(1, H * DK)),
        # [p, kt*1024+c] = w0[kt*128+p, c]
        "w0_t": cb(w0a.reshape(NKT, P, D).transpose(1, 0, 2).reshape(P, NKT * D)),
        "b0": cb(np.asarray(b0, f32).reshape(1, D)),
        # [ft, p, kt*128+c] = ff1_w[kt*128+p, ft*128+c]
        "f1_t": cb(f1a.reshape(NKT, P, NFT, P).transpose(2, 1, 0, 3)
                   .reshape(NFT, P, NKT * P)),
        "ff1_bT": c(np.asarray(ff1_b, f32).reshape(DFF, 1)),
        # [p, ft*1024+d] = ff2_w[ft*128+p, d]
        "f2_t": cb(f2a.reshape(NFT, P, D).transpose(1, 0, 2).reshape(P, NFT * D)),
        "ff2_b": cb(np.asarray(ff2_b, f32).reshape(1, D)),
        "ln1_g": c(np.asarray(ln1_g, f32).reshape(1, D)),
        "ln1_b": c(np.asarray(ln1_b, f32).reshape(1, D)),
        "ln2_g": c(np.asarray(ln2_g, f32).reshape(1, D)),
        "ln2_b": c(np.asarray(ln2_b, f32).reshape(1, D)),
    }
    in_maps = []
    for core in range(8):
        b, half = core // 2, core % 2
        rows = slice(half * SQ, (half + 1) * SQ)
        xqb = np.asarray(x_q[b], f32)[rows]            # [512, 1024]
        xkb = np.asarray(x_k[b], f32)                  # [1024, 1024]
        xvb = np.asarray(x_v[b], f32)
        m = dict(shared)
        # [p, kt*512+c] = x_q[b, half*512+c, kt*128+p]
        m["xq_t"] = cb(xqb.reshape(SQ, NKT, P).transpose(2, 1, 0)
                       .reshape(P, NKT * SQ))
        m["x_q_r"] = c(xqb)
        # [p, kt*1024+c] = x_k[b, c, kt*128+p]
        m["xk_t"] = cb(xkb.reshape(S, NKT, P).transpose(2, 1, 0)
                       .reshape(P, NKT * S))
        # [km, p, kt*128+c] = x_v[b, km*128+c, kt*128+p]
        m["xv_t"] = cb(xvb.reshape(NST, P, NKT, P).transpose(0, 3, 2, 1)
                       .reshape(NST, P, NKT * P))
        m["maskT"] = c(np.asarray(mask[b], f32).reshape(1, S).T)
        in_maps.append(m)
    return in_maps


def kernel(x_v, x_k, x_q, mask, wq, bq, wk, bk, wv, bv, w0, b0,
           ln1_g, ln1_b, ff1_w, ff1_b, ff2_w, ff2_b, ln2_g, ln2_b,
           _trace=False):
    from concourse import bass_utils

    flags = make_flags(mask, bq, bk, bv, b0, ff1_b, ff2_b,
                       ln1_g, ln1_b, ln2_g, ln2_b)
    nc = _get_program(flags)
    in_maps = make_in_maps(x_v, x_k, x_q, mask, wq, bq, wk, bk, wv, bv,
                           w0, b0, ln1_g, ln1_b, ff1_w, ff1_b,
                           ff2_w, ff2_b, ln2_g, ln2_b)
    res = bass_utils.run_bass_kernel_spmd(
        nc, in_maps, core_ids=list(range(8)), trace=_trace)

    out = np.empty((B, S, D), np.float32)
    attn = np.empty((B, S, S), np.float32)
    for core in range(8):
        b, half = core // 2, core % 2
        rows = slice(half * SQ, (half + 1) * SQ)
        out[b, rows] = res.results[core]["out_r"]
        attn[b, rows] = res.results[core]["attnw15"]
    if _trace:
        kernel.last_exec_time_ns = res.exec_time_ns
        kernel.last_trace = (res.instructions_and_trace or (None, None))[1]
        kernel.last_insts = (res.instructions_and_trace or (None, None))[0]
    return out, attn



# revision 12
# speedup vs baseline: 1.1549x; 1.1549x over previous
"""Trainium2 Bass kernel for a dense transformer encoder layer.

Problem shapes: B=4, S=1024, D=1024, H=16, DK=64, DFF=4096 (f32 I/O).
Returns (out [B,S,D], attn_w_last_head [B,S,S]) like the reference.

Sharding: 8 cores = 4 batches x 2 sequence halves. Each core computes
512 query rows of one batch end-to-end (K/V projections for the full
sequence are duplicated within each pair) -- no collectives.

All matmul operands are bf16 (fp32 PSUM accumulation); residual adds,
layernorms and outputs stay fp32.  Dataflow on each core:

  V[s,hdk]   = x_vT.T @ wv   (+ones col per head -> softmax denominator)
  QT[h],KT[h] per 128-feature M-tile (2 heads), padded to K=128
  per head (software-pipelined on PE, exp on ACT):
    sT[h]  = KT[h].T @ QT[h]           [1024, 512]  scores transposed
    eT[h]  = exp(sT + mask)            bf16 (ACT, 2 key-tiles per op)
    aT[h]  = (V'[h]).T @ eT[h]         [65, 512]  (row 64 = denom)
    concatT[h cols] = aT[0:64] * bcast(recip(denom))  (GpSimd bcast)
  mha[q,:]  = concatT.T @ w0 (+b0) + x_q -> LN1 -> sub1
  attn_w15  = exp(Q15.T K15) * recip(rowsum)  (recomputed q-major, f32)
  sub1T (PE transpose, bf16) ; hT[f] = relu(ff1_w.T @ sub1T)  [4096,512]
  ffn[q,:]  = sum_f hT[f].T @ ff2_w[f,:] (+ff2_b) ; +sub1 -> LN2 -> out

Weights/activations are pre-tiled AND pre-cast to bf16 on the host so
every SBUF tile loads as one DMA of contiguous per-partition rows.
"""

import numpy as np

B, S, D, H, DK, DFF = 4, 1024, 1024, 16, 64, 4096
EPS = 1e-6
P = 128
SQ = 512          # query rows per core
NQT = SQ // P     # 4 query-row tiles
NKT = D // P      # 8 contraction tiles over D
NST = S // P      # 8 key tiles
NFT = DFF // P    # 32 dff tiles


def _layernorm(nc, pool, x, eps_t, g_sb, b_sb, out_dma=None):
    """In-place layernorm over the free dim (D=1024) of x [128, 1024].

    With out_dma set (and no affine), the final normalize is split into
    halves with the output DMA issued per half to shorten the tail."""
    from concourse import mybir
    AF = mybir.ActivationFunctionType
    ALU = mybir.AluOpType
    F32 = mybir.dt.float32
    xr = x.rearrange("p (a b) -> p a b", b=512)
    stats = pool.tile([P, 2, 6], F32, tag="lnstats", name="lnstats")
    for i in range(2):
        nc.vector.bn_stats(stats[:, i, :], xr[:, i, :])
    mv = pool.tile([P, 2], F32, tag="lnmv", name="lnmv")
    nc.vector.bn_aggr(mv, stats)
    rstd = pool.tile([P, 1], F32, tag="lnrstd", name="lnrstd")
    nc.scalar.activation(rstd, mv[:, 1:2], AF.Sqrt, bias=eps_t)
    nc.vector.reciprocal(rstd, rstd)
    if out_dma is not None and g_sb is None and b_sb is None:
        for hh in range(2):
            sl = slice(hh * 512, hh * 512 + 512)
            nc.vector.tensor_scalar(out=x[:, sl], in0=x[:, sl],
                                    scalar1=mv[:, 0:1], scalar2=rstd,
                                    op0=ALU.subtract, op1=ALU.mult)
            nc.sync.dma_start(out_dma[:, sl], x[:, sl])
        return
    nc.vector.tensor_scalar(out=x, in0=x, scalar1=mv[:, 0:1], scalar2=rstd,
                            op0=ALU.subtract, op1=ALU.mult)
    if g_sb is not None:
        nc.vector.tensor_mul(x, x, g_sb)
    if b_sb is not None:
        nc.vector.tensor_add(x, x, b_sb)
    if out_dma is not None:
        nc.sync.dma_start(out_dma, x)


def _build(flags):
    import concourse.bass as bass
    import concourse.tile as tile
    from concourse import bacc, mybir
    from concourse.masks import make_identity
    from contextlib import ExitStack

    dt = mybir.dt
    AF = mybir.ActivationFunctionType
    ALU = mybir.AluOpType
    F32, BF16 = dt.float32, dt.bfloat16

    nc = bacc.Bacc("TRN2", target_bir_lowering=False, debug=False)

    def din(name, shape, dtp=BF16):
        return nc.dram_tensor(name, shape, dtp, kind="ExternalInput").ap()

    def dout(name, shape):
        return nc.dram_tensor(name, shape, F32, kind="ExternalOutput").ap()

    # pre-tiled inputs (see make_in_maps for layouts)
    xq_t = din("xq_t", (P, NKT * SQ))
    xk_t = din("xk_t", (P, NKT * S))
    xv_t = din("xv_t", (NST, P, NKT * P))
    x_q_r = din("x_q_r", (SQ, D))
    maskT = din("maskT", (S, 1), F32)
    wq_t = din("wq_t", (NKT, P, NKT * P))
    wk_t = din("wk_t", (NKT, P, NKT * P))
    wv_t = din("wv_t", (2, P, NKT * 512))
    bq2T = din("bq2T", (H * DK, 1), F32)
    bk2T = din("bk2T", (H * DK, 1), F32)
    bv2 = din("bv2", (1, H * DK))
    w0_t = din("w0_t", (P, NKT * D))
    b0 = din("b0", (1, D))
    f1_t = din("f1_t", (NFT, P, NKT * P))
    ff1_bT = din("ff1_bT", (DFF, 1), F32)
    f2_t = din("f2_t", (P, NFT * D))
    ff2_b = din("ff2_b", (1, D))
    ln1_g = din("ln1_g", (1, D), F32)
    ln1_b = din("ln1_b", (1, D), F32)
    ln2_g = din("ln2_g", (1, D), F32)
    ln2_b = din("ln2_b", (1, D), F32)

    out_r = dout("out_r", (SQ, D))
    attnw15 = dout("attnw15", (SQ, S))

    def bcast_row(ap_1xn):
        return bass.AP(tensor=ap_1xn.tensor, offset=ap_1xn.offset,
                       ap=[[0, P]] + list(ap_1xn.ap[1:]))

    def flat(t3):
        return t3.rearrange("p a b -> p (a b)")

    with tile.TileContext(nc) as tc, ExitStack() as top:
        consts = top.enter_context(tc.tile_pool(name="consts", bufs=1))
        persist = top.enter_context(tc.tile_pool(name="persist", bufs=1))

        ident = consts.tile([P, P], F32)
        eps_t = consts.tile([P, 1], F32)
        ones_row = None
        if flags["b0"] or flags["bv"] or flags["mask"] or flags["ff2b"]:
            ones_row = consts.tile([1, P], BF16)
            nc.gpsimd.memset(ones_row, 1.0)

        ln1g_sb = ln1b_sb = ln2g_sb = ln2b_sb = None
        if flags["mask"]:
            mrow = consts.tile([P, NST], F32)
            nc.sync.dma_start(mrow, maskT.rearrange("(t p) o -> p (t o)", p=P))
            nc.scalar.mul(mrow, mrow, -1e9)
            # bf16 masked row for the q-major attnw15 recompute
            mrow_f = consts.tile([1, S], F32)
            nc.sync.dma_start(mrow_f, maskT.rearrange("s o -> o s"))
            nc.scalar.mul(mrow_f, mrow_f, -1e9)
            mrow_r = consts.tile([1, S], BF16)
            nc.vector.tensor_copy(mrow_r, mrow_f)
        if flags["b0"]:
            b0_sb = consts.tile([1, D], BF16)
            nc.sync.dma_start(b0_sb, b0)
        if flags["bv"]:
            bv2_sb = consts.tile([1, H * DK], BF16)
            nc.sync.dma_start(bv2_sb, bv2)
        if flags["bq"]:
            bqT_sb = consts.tile([P, NST], F32)
            nc.sync.dma_start(bqT_sb, bq2T.rearrange("(t p) o -> p (t o)", p=P))
        if flags["bk"]:
            bkT_sb = consts.tile([P, NST], F32)
            nc.sync.dma_start(bkT_sb, bk2T.rearrange("(t p) o -> p (t o)", p=P))
        if flags["ff1b"]:
            f1bT_sb = consts.tile([P, NFT], F32)
            nc.sync.dma_start(f1bT_sb, ff1_bT.rearrange("(t p) o -> p (t o)", p=P))
        if flags["ff2b"]:
            f2b_sb = consts.tile([1, D], BF16)
            nc.sync.dma_start(f2b_sb, ff2_b)
        if flags["ln1g"]:
            ln1g_sb = consts.tile([P, D], F32, tag="ln1g")
            nc.sync.dma_start(ln1g_sb, bcast_row(ln1_g))
        if flags["ln1b"]:
            ln1b_sb = consts.tile([P, D], F32, tag="ln1b")
            nc.sync.dma_start(ln1b_sb, bcast_row(ln1_b))
        if flags["ln2g"]:
            ln2g_sb = consts.tile([P, D], F32, tag="ln2g")
            nc.sync.dma_start(ln2g_sb, bcast_row(ln2_g))
        if flags["ln2b"]:
            ln2b_sb = consts.tile([P, D], F32, tag="ln2b")
            nc.sync.dma_start(ln2b_sb, bcast_row(ln2_b))

        # transposed, normalized attention output [cin, q] (bf16)
        concatT = persist.tile([P, NKT, SQ], BF16, tag="concatT", name="concatT")
        # w0 lives in the top scope so its DMA can issue during the attention
        # phase; w1 streams with a prefetch ring that starts during W0
        w0_sb = persist.tile([P, NKT, D], BF16, tag="w0t")
        w1pool = top.enter_context(tc.tile_pool(name="w1p", bufs=6))
        w1_tiles = {}

        def w1_fetch(ft):
            w1 = w1pool.tile([P, NKT, P], BF16, tag="w1t", name=f"w1_{ft}")
            nc.sync.dma_start(flat(w1), f1_t[ft])
            w1_tiles[ft] = w1

        # ---------------- attention: single pass, 16 heads ----------------
        attn_scope = top.enter_context(ExitStack())
        hpool = attn_scope.enter_context(tc.tile_pool(name="hs", bufs=1))
        # q/k per-head tiles, zero-padded to K=128 partitions
        qt_pad = hpool.tile([P, H, SQ], BF16, tag="qt")
        kt_pad = hpool.tile([P, H, S], BF16, tag="kt")
        v_sb = hpool.tile([P, NST, H, DK + 1], BF16, tag="v")

        with ExitStack() as ph:
            xpool = ph.enter_context(tc.tile_pool(name="xs", bufs=1))
            vscope = ExitStack()
            vxpool = vscope.enter_context(tc.tile_pool(name="vx", bufs=1))

            # kick off all attention DMAs before any on-chip init so the
            # first V matmul can start as early as possible
            wv_sb = vxpool.tile([P, 2, NKT, 512], BF16, tag="wv")
            wvf = wv_sb.rearrange("p h a b -> p (h a b)")
            nc.sync.dma_start(wvf[:, 0:4 * 512], wv_t[0][:, 0:4 * 512])
            nc.sync.dma_start(wvf[:, 4 * 512:NKT * 512], wv_t[0][:, 4 * 512:])
            xv_ts = []
            for km in range(NST):
                xv = vxpool.tile([P, NKT, P], BF16, tag=f"xv{km}",
                                 name=f"xv{km}")
                nc.sync.dma_start(flat(xv), xv_t[km])
                xv_ts.append(xv)
            nc.sync.dma_start(wv_sb[:, 1].rearrange("p a b -> p (a b)"), wv_t[1])
            xq_sb = xpool.tile([P, NKT, SQ], BF16, tag="xq")
            nc.sync.dma_start(flat(xq_sb), xq_t)
            xk_sb = xpool.tile([P, NKT, S], BF16, tag="xk")
            nc.sync.dma_start(flat(xk_sb)[:, 0:NKT * 512], xk_t[:, 0:NKT * 512])
            nc.sync.dma_start(flat(xk_sb)[:, NKT * 512:], xk_t[:, NKT * 512:])

            # on-chip constant init on the (otherwise idle) GpSimd engine
            make_identity(nc, ident)
            nc.gpsimd.memset(eps_t, EPS)
            nc.gpsimd.memset(qt_pad[64:128, :, :], 0.0)
            nc.gpsimd.memset(kt_pad[64:128, :, :], 0.0)
            nc.vector.memset(v_sb[:, :, :, DK:DK + 1], 1.0)

            pp = ph.enter_context(tc.tile_pool(name="pp", bufs=2, space="PSUM"))
            ps = ph.enter_context(tc.tile_pool(name="ps", bufs=2, space="PSUM"))
            pa = ph.enter_context(tc.tile_pool(name="pa", bufs=2, space="PSUM"))

            # V projection first: all 16 heads, hdk-half outer so compute can
            # begin as soon as the first wv half lands
            for hf in range(2):
                for km in range(NST):
                    pv = pp.tile([P, 512], F32, tag="pp", name="pv")
                    for kt in range(NKT):
                        nc.tensor.matmul(pv, xv_ts[km][:, kt, :],
                                         wv_sb[:, hf, kt, :],
                                         start=(kt == 0),
                                         stop=(kt == NKT - 1 and not flags["bv"]))
                    if flags["bv"]:
                        nc.tensor.matmul(pv, ones_row,
                                         bv2_sb[:, hf * 512:hf * 512 + 512],
                                         start=False, stop=True)
                    nc.vector.tensor_copy(
                        v_sb[:, km, hf * 8:hf * 8 + 8, 0:DK],
                        pv.rearrange("p (s c) -> p s c", c=DK))
            # wv / xv space is dead from here on -- release it for the
            # eT / recip pools
            vscope.close()
            wpool = ph.enter_context(tc.tile_pool(name="w", bufs=5))
            epool = ph.enter_context(tc.tile_pool(name="e", bufs=3))
            rpool = ph.enter_context(tc.tile_pool(name="r", bufs=2))

            # q/k weight tiles stream ahead of the projection loop (2-deep
            # prefetch); w0 queues behind them -- it is not needed until the
            # attention phase ends
            qk_tiles = {}

            def qk_fetch(mt):
                wq_mt = wpool.tile([P, NKT, P], BF16, tag="wq_mt",
                                   name=f"wq{mt}")
                nc.sync.dma_start(flat(wq_mt), wq_t[mt])
                wk_mt = wpool.tile([P, NKT, P], BF16, tag="wk_mt",
                                   name=f"wk{mt}")
                nc.sync.dma_start(flat(wk_mt), wk_t[mt])
                qk_tiles[mt] = (wq_mt, wk_mt)

            qk_fetch(0)
            qk_fetch(1)
            nc.sync.dma_start(flat(w0_sb), w0_t)

            def scores_head(h):
                """8 score matmuls + exp into a fresh eT tile; returns eT."""
                eT = epool.tile([P, NST, SQ], BF16, tag="eT", name=f"eT{h}")
                for g in range(4):          # pairs of key tiles
                    psc = ps.tile([P, 2, 512], F32, tag="psc", name="psc")
                    for j in range(2):
                        st = 2 * g + j
                        nc.tensor.matmul(
                            psc[:, j, :],
                            kt_pad[:, h, st * P:(st + 1) * P],
                            qt_pad[:, h, :], start=True, stop=True)
                    if flags["mask"]:
                        for j in range(2):
                            st = 2 * g + j
                            nc.scalar.activation(
                                eT[:, st, :], psc[:, j, :], AF.Exp,
                                bias=mrow[:, st:st + 1])
                    else:
                        nc.scalar.activation(
                            eT[:, 2 * g:2 * g + 2, :], psc, AF.Exp)
                return eT

            def attnv_mm(h, eT):
                """aT = V'.T @ eT (row 64 = softmax denominator).  The
                softmax normalization is fused into the PSUM eviction: a
                per-head reciprocal of the denominator row is broadcast
                across partitions on GpSimd, then one tensor_tensor multiply
                evicts pat into concatT already normalized."""
                pat = pa.tile([DK + 1, SQ], F32, tag="pat", name="pat")
                for st in range(NST):
                    nc.tensor.matmul(pat, v_sb[:, st, h, :], eT[:, st, :],
                                     start=(st == 0), stop=(st == NST - 1))
                rjf = rpool.tile([1, SQ], F32, tag="rjf", name=f"rjf{h}",
                                 bufs=2)
                nc.vector.tensor_copy(rjf, pat[DK:DK + 1, :])
                nc.vector.reciprocal(rjf, rjf)
                rj = rpool.tile([1, SQ], BF16, tag="rj", name=f"rj{h}",
                                bufs=2)
                nc.vector.tensor_copy(rj, rjf)
                rb = rpool.tile([P, SQ], BF16, tag="rb", name=f"rb{h}",
                                bufs=3)
                nc.gpsimd.partition_broadcast(rb, rj, channels=P)
                po = (h % 2) * 64
                nc.vector.tensor_tensor(
                    out=concatT[po:po + 64, h // 2, :], in0=pat[0:DK, :],
                    in1=rb[po:po + 64, :], op=ALU.mult)

            # software pipeline: Q/K proj of tile mt, scores(h), attnV
            # matmuls of h-2.
            pending = []
            for mt in range(NKT):
                if mt + 2 < NKT:
                    qk_fetch(mt + 2)
                wq_mt, wk_mt = qk_tiles.pop(mt)
                pq = pp.tile([P, 512], F32, tag="pp", name="pq")
                for kt in range(NKT):
                    nc.tensor.matmul(pq, wq_mt[:, kt, :], xq_sb[:, kt, :],
                                     start=(kt == 0), stop=(kt == NKT - 1))
                for sub in range(2):
                    psl = slice(sub * 64, sub * 64 + 64)
                    # Q evictions ride the scalar engine to keep the DVE
                    # FIFO clear for the K / attnV eviction chain
                    nc.scalar.activation(
                        qt_pad[0:64, 2 * mt + sub, :], pq[psl, :],
                        AF.Identity,
                        bias=bqT_sb[psl, mt:mt + 1] if flags["bq"] else 0.0)

                for half in range(2):
                    fsl = slice(half * 512, half * 512 + 512)
                    pk = pp.tile([P, 512], F32, tag="pp", name="pk")
                    for kt in range(NKT):
                        nc.tensor.matmul(pk, wk_mt[:, kt, :],
                                         xk_sb[:, kt, fsl],
                                         start=(kt == 0), stop=(kt == NKT - 1))
                    for sub in range(2):
                        psl = slice(sub * 64, sub * 64 + 64)
                        if flags["bk"]:
                            nc.scalar.activation(
                                kt_pad[0:64, 2 * mt + sub, fsl], pk[psl, :],
                                AF.Identity, bias=bkT_sb[psl, mt:mt + 1])
                        else:
                            nc.vector.tensor_copy(
                                kt_pad[0:64, 2 * mt + sub, fsl], pk[psl, :])

                for h in (2 * mt, 2 * mt + 1):
                    eT = scores_head(h)
                    if len(pending) == 2:
                        hh, eTT = pending.pop(0)
                        attnv_mm(hh, eTT)
                    pending.append((h, eT))
            for hh, eTT in pending:
                attnv_mm(hh, eTT)

        # ---------------- mha out + residual + LN1 + attnw15 --------------
        sub1 = [persist.tile([P, D], F32, tag=f"sub1_{qt}", name=f"sub1_{qt}")
                for qt in range(NQT)]
        sub1T = persist.tile([P, NKT, SQ], BF16, tag="sub1T")
        with ExitStack() as ph:
            xpool = ph.enter_context(tc.tile_pool(name="xr", bufs=2))
            apool = ph.enter_context(tc.tile_pool(name="a15", bufs=2))
            lnpool = ph.enter_context(tc.tile_pool(name="ln1pool", bufs=4))
            po = ph.enter_context(tc.tile_pool(name="po", bufs=2, space="PSUM"))
            p15 = ph.enter_context(tc.tile_pool(name="p15", bufs=2, space="PSUM"))
            pt = ph.enter_context(tc.tile_pool(name="ptr", bufs=2, space="PSUM"))

            # start the FF1 weight stream while W0 runs
            for ft in range(6):
                w1_fetch(ft)

            def w0_block(qt):
                # kt outer / half inner: both halves stream from the same
                # stationary concatT tile, halving the weight switches
                pmo = po.tile([P, 2, 512], F32, tag="pmo", name="pmo")
                for kt in range(NKT):
                    for half in range(2):
                        fsl = slice(half * 512, half * 512 + 512)
                        nc.tensor.matmul(pmo[:, half, :],
                                         concatT[:, kt, qt * P:(qt + 1) * P],
                                         w0_sb[:, kt, fsl],
                                         start=(kt == 0),
                                         stop=(kt == NKT - 1 and not flags["b0"]))
                if flags["b0"]:
                    for half in range(2):
                        fsl = slice(half * 512, half * 512 + 512)
                        nc.tensor.matmul(pmo[:, half, :], ones_row,
                                         b0_sb[:, fsl], start=False, stop=True)
                xq = xpool.tile([P, D], BF16, tag="xqr")
                nc.sync.dma_start(xq, x_q_r[qt * P:(qt + 1) * P, :])
                nc.vector.tensor_add(sub1[qt], flat(pmo), xq)
                _layernorm(nc, lnpool, sub1[qt], eps_t, ln1g_sb, ln1b_sb)

            def t_block(qt):
                # transpose sub1 (f32) into sub1T [c, q] (bf16 on evict)
                for ct in range(NKT):
                    ptt = pt.tile([P, P], F32, tag="ptt", name="ptt")
                    nc.tensor.transpose(
                        ptt, sub1[qt][:, ct * P:(ct + 1) * P], ident)
                    nc.vector.tensor_copy(
                        sub1T[:, ct, qt * P:(qt + 1) * P], ptt)

            # interleave so each transpose block's LN1 is ready when the PE
            # reaches it
            w0_block(0)
            for qt in range(1, NQT):
                w0_block(qt)
                t_block(qt - 1)
            t_block(NQT - 1)

            # head-15 attention weights: recompute scores q-major (f32 path).
            # De-prioritized PE/ACT filler; the row-sum + normalize runs on
            # GpSimd (otherwise idle here) so the DVE stays clear for LN1 and
            # the sub1T transpose evictions.
            with tc.high_priority(offset=-180):
                for qt in range(NQT):
                    a15 = apool.tile([P, S], F32, tag="a15")
                    a15n = apool.tile([P, S], F32, tag="a15n", name="a15n")
                    for half in range(2):
                        fsl = slice(half * 512, half * 512 + 512)
                        pw = p15.tile([P, 512], F32, tag="p15", name="pw")
                        nc.tensor.matmul(
                            pw, qt_pad[:, H - 1, qt * P:(qt + 1) * P],
                            kt_pad[:, H - 1, fsl],
                            start=True, stop=not flags["mask"])
                        if flags["mask"]:
                            nc.tensor.matmul(pw, ones_row, mrow_r[:, fsl],
                                             start=False, stop=True)
                        nc.scalar.activation(a15[:, fsl], pw, AF.Exp)
                    den = apool.tile([P, 1], F32, tag="den15", name="den15")
                    nc.vector.tensor_reduce(den, a15, mybir.AxisListType.X,
                                            ALU.add)
                    nc.gpsimd.normalize_recip(a15n, a15, den)
                    nc.sync.dma_start(attnw15[qt * P:(qt + 1) * P, :], a15n)

        attn_scope.close()

        # ---------------- FFN + residual + LN2 ----------------
        with ExitStack() as ph:
            mpool = ph.enter_context(tc.tile_pool(name="f", bufs=1))
            w2pool = ph.enter_context(tc.tile_pool(name="w2p", bufs=1))
            lnpool = ph.enter_context(tc.tile_pool(name="ln2pool", bufs=4))
            pf = ph.enter_context(tc.tile_pool(name="pf", bufs=3, space="PSUM"))
            pg = ph.enter_context(tc.tile_pool(name="pg", bufs=2, space="PSUM"))

            w2_sb = w2pool.tile([P, NFT, D], BF16, tag="w2t")

            hT = mpool.tile([P, NFT, SQ], BF16, tag="hT")
            for ft in range(NFT):
                if ft + 6 < NFT:
                    w1_fetch(ft + 6)
                if ft == 16:
                    # FF2 weights: issued midway through FF1 so they queue
                    # behind the remaining FF1 stream but land before FF2
                    for c in range(8):
                        nc.sync.dma_start(
                            w2_sb[:, 4 * c:4 * c + 4, :],
                            f2_t.rearrange("p (a b) -> p a b", b=D)
                            [:, 4 * c:4 * c + 4, :])
                w1 = w1_tiles.pop(ft)
                pff = pf.tile([P, SQ], F32, tag="pff", name="pff")
                for kt in range(NKT):
                    nc.tensor.matmul(pff, w1[:, kt, :], sub1T[:, kt, :],
                                     start=(kt == 0), stop=(kt == NKT - 1))
                if flags["ff1b"]:
                    nc.vector.tensor_scalar(
                        out=hT[:, ft, :], in0=pff,
                        scalar1=f1bT_sb[:, ft:ft + 1], scalar2=0.0,
                        op0=ALU.add, op1=ALU.max)
                else:
                    nc.vector.tensor_scalar_max(
                        out=hT[:, ft, :], in0=pff, scalar1=0.0)

            for qt in range(NQT):
                # ft outer / half inner: both halves stream from the same
                # stationary hT tile, halving the weight switches
                pfn = pg.tile([P, 2, 512], F32, tag="pfn", name="pfn")
                for ft in range(NFT):
                    for half in range(2):
                        fsl = slice(half * 512, half * 512 + 512)
                        nc.tensor.matmul(
                            pfn[:, half, :],
                            hT[:, ft, qt * P:(qt + 1) * P],
                            w2_sb[:, ft, fsl],
                            start=(ft == 0),
                            stop=(ft == NFT - 1 and not flags["ff2b"]))
                if flags["ff2b"]:
                    for half in range(2):
                        fsl = slice(half * 512, half * 512 + 512)
                        nc.tensor.matmul(pfn[:, half, :], ones_row,
                                         f2b_sb[:, fsl], start=False, stop=True)
                nc.vector.tensor_add(sub1[qt], flat(pfn), sub1[qt])
                _layernorm(nc, lnpool, sub1[qt], eps_t, ln2g_sb, ln2b_sb,
                           out_dma=out_r[qt * P:(qt + 1) * P, :])

    nc.compile()
    return nc


_CACHE = {}


def _get_program(flags):
    key = tuple(sorted(flags.items()))
    if key not in _CACHE:
        _CACHE[key] = _build(flags)
    return _CACHE[key]


def make_flags(mask, bq, bk, bv, b0, ff1_b, ff2_b, ln1_g, ln1_b, ln2_g, ln2_b):
    return {
        "mask": bool(np.any(mask)), "bq": bool(np.any(bq)),
        "bk": bool(np.any(bk)), "bv": bool(np.any(bv)),
        "b0": bool(np.any(b0)), "ff1b": bool(np.any(ff1_b)),
        "ff2b": bool(np.any(ff2_b)),
        "ln1g": bool(np.any(ln1_g != 1.0)), "ln1b": bool(np.any(ln1_b)),
        "ln2g": bool(np.any(ln2_g != 1.0)), "ln2b": bool(np.any(ln2_b)),
    }


def make_in_maps(x_v, x_k, x_q, mask, wq, bq, wk, bk, wv, bv, w0, b0,
                 ln1_g, ln1_b, ff1_w, ff1_b, ff2_w, ff2_b, ln2_g, ln2_b):
    import ml_dtypes
    f32 = np.float32
    bf16 = ml_dtypes.bfloat16
    c = np.ascontiguousarray

    def cb(a):
        return c(np.asarray(a, f32).astype(bf16))

    wq2 = np.transpose(np.asarray(wq, f32), (1, 0, 2)).reshape(D, H * DK) / 8.0
    wk2 = np.transpose(np.asarray(wk, f32), (1, 0, 2)).reshape(D, H * DK)
    wv2 = np.transpose(np.asarray(wv, f32), (1, 0, 2)).reshape(D, H * DK)
    w0a = np.asarray(w0, f32)
    f1a = np.asarray(ff1_w, f32)
    f2a = np.asarray(ff2_w, f32)
    shared = {
        # [mt, p, kt*128+c] = wq2[kt*128+p, mt*128+c]
        "wq_t": cb(wq2.reshape(NKT, P, NKT, P).transpose(2, 1, 0, 3)
                   .reshape(NKT, P, NKT * P)),
        "wk_t": cb(wk2.reshape(NKT, P, NKT, P).transpose(2, 1, 0, 3)
                   .reshape(NKT, P, NKT * P)),
        # [hf, p, kt*512+c] = wv2[kt*128+p, hf*512+c]
        "wv_t": cb(wv2.reshape(NKT, P, 2, 512).transpose(2, 1, 0, 3)
                   .reshape(2, P, NKT * 512)),
        "bq2T": c(np.asarray(bq, f32).reshape(H * DK, 1) / 8.0),
        "bk2T": c(np.asarray(bk, f32).reshape(H * DK, 1)),
        "bv2": cb(np.asarray(bv, f32).reshape(1, H * DK)),
        # [p, kt*1024+c] = w0[kt*128+p, c]
        "w0_t": cb(w0a.reshape(NKT, P, D).transpose(1, 0, 2).reshape(P, NKT * D)),
        "b0": cb(np.asarray(b0, f32).reshape(1, D)),
        # [ft, p, kt*128+c] = ff1_w[kt*128+p, ft*128+c]
        "f1_t": cb(f1a.reshape(NKT, P, NFT, P).transpose(2, 1, 0, 3)
                   .reshape(NFT, P, NKT * P)),
        "ff1_bT": c(np.asarray(ff1_b, f32).reshape(DFF, 1)),
        # [p, ft*1024+d] = ff2_w[ft*128+p, d]
        "f2_t": cb(f2a.reshape(NFT, P, D).transpose(1, 0, 2).reshape(P, NFT * D)),
        "ff2_b": cb(np.asarray(ff2_b, f32).reshape(1, D)),
        "ln1_g": c(np.asarray(ln1_g, f32).reshape(1, D)),
        "ln1_b": c(np.asarray(ln1_b, f32).reshape(1, D)),
        "ln2_g": c(np.asarray(ln2_g, f32).reshape(1, D)),
        "ln2_b": c(np.asarray(ln2_b, f32).reshape(1, D)),
    }
    in_maps = []
    for core in range(8):
        b, half = core // 2, core % 2
        rows = slice(half * SQ, (half + 1) * SQ)
        xqb = np.asarray(x_q[b], f32)[rows]            # [512, 1024]
        xkb = np.asarray(x_k[b], f32)                  # [1024, 1024]
        xvb = np.asarray(x_v[b], f32)
        m = dict(shared)
        # [p, kt*512+c] = x_q[b, half*512+c, kt*128+p]
        m["xq_t"] = cb(xqb.reshape(SQ, NKT, P).transpose(2, 1, 0)
                       .reshape(P, NKT * SQ))
        m["x_q_r"] = cb(xqb)
        # [p, kt*1024+c] = x_k[b, c, kt*128+p]
        m["xk_t"] = cb(xkb.reshape(S, NKT, P).transpose(2, 1, 0)
                       .reshape(P, NKT * S))
        # [km, p, kt*128+c] = x_v[b, km*128+c, kt*128+p]
        m["xv_t"] = cb(xvb.reshape(NST, P, NKT, P).transpose(0, 3, 2, 1)
                       .reshape(NST, P, NKT * P))
        m["maskT"] = c(np.asarray(mask[b], f32).reshape(1, S).T)
        in_maps.append(m)
    return in_maps


def kernel(x_v, x_k, x_q, mask, wq, bq, wk, bk, wv, bv, w0, b0,
           ln1_g, ln1_b, ff1_w, ff1_b, ff2_w, ff2_b, ln2_g, ln2_b,
           _trace=False):
    from concourse import bass_utils

    flags = make_flags(mask, bq, bk, bv, b0, ff1_b, ff2_b,
                       ln1_g, ln1_b, ln2_g, ln2_b)
    nc = _get_program(flags)
    in_maps = make_in_maps(x_v, x_k, x_q, mask, wq, bq, wk, bk, wv, bv,
                           w0, b0, ln1_g, ln1_b, ff1_w, ff1_b,
                           ff2_w, ff2_b, ln2_g, ln2_b)
    res = bass_utils.run_bass_kernel_spmd(
        nc, in_maps, core_ids=list(range(8)), trace=_trace)

    out = np.empty((B, S, D), np.float32)
    attn = np.empty((B, S, S), np.float32)
    for core in range(8):
        b, half = core // 2, core % 2
        rows = slice(half * SQ, (half + 1) * SQ)
        out[b, rows] = res.results[core]["out_r"]
        attn[b, rows] = res.results[core]["attnw15"]
    if _trace:
        kernel.last_exec_time_ns = res.exec_time_ns
        kernel.last_trace = (res.instructions_and_trace or (None, None))[1]
        kernel.last_insts = (res.instructions_and_trace or (None, None))[0]
    return out, attn



# revision 13
# speedup vs baseline: 1.3255x; 1.1477x over previous
"""Trainium2 Bass kernel for a dense transformer encoder layer.

Problem shapes: B=4, S=1024, D=1024, H=16, DK=64, DFF=4096 (f32 I/O).
Returns (out [B,S,D], attn_w_last_head [B,S,S]) like the reference.

Sharding: 8 cores = 4 batches x 2 sequence halves. Each core computes
512 query rows of one batch end-to-end (K/V projections for the full
sequence are duplicated within each pair) -- no collectives.

All matmul operands are bf16 (fp32 PSUM accumulation); residual adds,
layernorms and outputs stay fp32.  Dataflow on each core:

  V[s,hdk]   = x_vT.T @ wv   (+ones col per head -> softmax denominator)
  QT[h],KT[h] per 128-feature M-tile (2 heads), padded to K=128
  per head (software-pipelined on PE, exp on ACT):
    sT[h]  = KT[h].T @ QT[h]           [1024, 512]  scores transposed
    eT[h]  = exp(sT + mask)            bf16 (ACT, 2 key-tiles per op)
    aT[h]  = (V'[h]).T @ eT[h]         [65, 512]  (row 64 = denom)
    concatT[h cols] = aT[0:64] * bcast(recip(denom))  (GpSimd bcast)
  mha[q,:]  = concatT.T @ w0 (+b0) + x_q -> LN1 -> sub1
  attn_w15  = exp(Q15.T K15) * recip(rowsum)  (recomputed q-major, f32)
  sub1T (PE transpose, bf16) ; hT[f] = relu(ff1_w.T @ sub1T)  [4096,512]
  ffn[q,:]  = sum_f hT[f].T @ ff2_w[f,:] (+ff2_b) ; +sub1 -> LN2 -> out

Weights/activations are pre-tiled AND pre-cast to bf16 on the host so
every SBUF tile loads as one DMA of contiguous per-partition rows.
"""

import numpy as np

B, S, D, H, DK, DFF = 4, 1024, 1024, 16, 64, 4096
EPS = 1e-6
P = 128
SQ = 512          # query rows per core
NQT = SQ // P     # 4 query-row tiles
NKT = D // P      # 8 contraction tiles over D
NST = S // P      # 8 key tiles
NFT = DFF // P    # 32 dff tiles


def _layernorm(nc, pool, x, eps_t, g_sb, b_sb, out_dma=None):
    """In-place layernorm over the free dim (D=1024) of x [128, 1024].

    With out_dma set (and no affine), the final normalize is split into
    halves with the output DMA issued per half to shorten the tail."""
    from concourse import mybir
    AF = mybir.ActivationFunctionType
    ALU = mybir.AluOpType
    F32 = mybir.dt.float32
    xr = x.rearrange("p (a b) -> p a b", b=512)
    stats = pool.tile([P, 2, 6], F32, tag="lnstats", name="lnstats")
    for i in range(2):
        nc.vector.bn_stats(stats[:, i, :], xr[:, i, :])
    mv = pool.tile([P, 2], F32, tag="lnmv", name="lnmv")
    nc.vector.bn_aggr(mv, stats)
    rstd = pool.tile([P, 1], F32, tag="lnrstd", name="lnrstd")
    nc.scalar.activation(rstd, mv[:, 1:2], AF.Sqrt, bias=eps_t)
    nc.vector.reciprocal(rstd, rstd)
    if out_dma is not None and g_sb is None and b_sb is None:
        for hh in range(2):
            sl = slice(hh * 512, hh * 512 + 512)
            nc.vector.tensor_scalar(out=x[:, sl], in0=x[:, sl],
                                    scalar1=mv[:, 0:1], scalar2=rstd,
                                    op0=ALU.subtract, op1=ALU.mult)
            nc.sync.dma_start(out_dma[:, sl], x[:, sl])
        return
    nc.vector.tensor_scalar(out=x, in0=x, scalar1=mv[:, 0:1], scalar2=rstd,
                            op0=ALU.subtract, op1=ALU.mult)
    if g_sb is not None:
        nc.vector.tensor_mul(x, x, g_sb)
    if b_sb is not None:
        nc.vector.tensor_add(x, x, b_sb)
    if out_dma is not None:
        nc.sync.dma_start(out_dma, x)


def _build(flags):
    import concourse.bass as bass
    import concourse.tile as tile
    from concourse import bacc, mybir
    from concourse.masks import make_identity
    from contextlib import ExitStack

    dt = mybir.dt
    AF = mybir.ActivationFunctionType
    ALU = mybir.AluOpType
    F32, BF16 = dt.float32, dt.bfloat16

    nc = bacc.Bacc("TRN2", target_bir_lowering=False, debug=False)

    def din(name, shape, dtp=BF16):
        return nc.dram_tensor(name, shape, dtp, kind="ExternalInput").ap()

    def dout(name, shape):
        return nc.dram_tensor(name, shape, F32, kind="ExternalOutput").ap()

    # pre-tiled inputs (see make_in_maps for layouts)
    xq_t = din("xq_t", (P, NKT * SQ))
    xk_t = din("xk_t", (P, NKT * S))
    xv_t = din("xv_t", (NST, P, NKT * P))
    x_q_r = din("x_q_r", (SQ, D))
    maskT = din("maskT", (S, 1), F32)
    wq_t = din("wq_t", (NKT, P, NKT * P))
    wk_t = din("wk_t", (NKT, P, NKT * P))
    wv_t = din("wv_t", (2, P, NKT * 512))
    bq2T = din("bq2T", (H * DK, 1), F32)
    bk2T = din("bk2T", (H * DK, 1), F32)
    bv2 = din("bv2", (1, H * DK))
    w0_t = din("w0_t", (P, NKT * D))
    b0 = din("b0", (1, D))
    f1_t = din("f1_t", (NFT, P, NKT * P))
    ff1_bT = din("ff1_bT", (DFF, 1), F32)
    f2_t = din("f2_t", (P, NFT * D))
    ff2_b = din("ff2_b", (1, D))
    ln1_g = din("ln1_g", (1, D), F32)
    ln1_b = din("ln1_b", (1, D), F32)
    ln2_g = din("ln2_g", (1, D), F32)
    ln2_b = din("ln2_b", (1, D), F32)

    out_r = dout("out_r", (SQ, D))
    attnw15 = dout("attnw15", (SQ, S))

    def bcast_row(ap_1xn):
        return bass.AP(tensor=ap_1xn.tensor, offset=ap_1xn.offset,
                       ap=[[0, P]] + list(ap_1xn.ap[1:]))

    def flat(t3):
        return t3.rearrange("p a b -> p (a b)")

    with tile.TileContext(nc) as tc, ExitStack() as top:
        consts = top.enter_context(tc.tile_pool(name="consts", bufs=1))
        persist = top.enter_context(tc.tile_pool(name="persist", bufs=1))

        ident = consts.tile([P, P], F32)
        eps_t = consts.tile([P, 1], F32)
        ones_row = None
        if flags["b0"] or flags["bv"] or flags["mask"] or flags["ff2b"]:
            ones_row = consts.tile([1, P], BF16)
            nc.gpsimd.memset(ones_row, 1.0)

        ln1g_sb = ln1b_sb = ln2g_sb = ln2b_sb = None
        if flags["mask"]:
            mrow = consts.tile([P, NST], F32)
            nc.sync.dma_start(mrow, maskT.rearrange("(t p) o -> p (t o)", p=P))
            nc.scalar.mul(mrow, mrow, -1e9)
            # bf16 masked row for the q-major attnw15 recompute
            mrow_f = consts.tile([1, S], F32)
            nc.sync.dma_start(mrow_f, maskT.rearrange("s o -> o s"))
            nc.scalar.mul(mrow_f, mrow_f, -1e9)
            mrow_r = consts.tile([1, S], BF16)
            nc.vector.tensor_copy(mrow_r, mrow_f)
        if flags["b0"]:
            b0_sb = consts.tile([1, D], BF16)
            nc.sync.dma_start(b0_sb, b0)
        if flags["bv"]:
            bv2_sb = consts.tile([1, H * DK], BF16)
            nc.sync.dma_start(bv2_sb, bv2)
        if flags["bq"]:
            bqT_sb = consts.tile([P, NST], F32)
            nc.sync.dma_start(bqT_sb, bq2T.rearrange("(t p) o -> p (t o)", p=P))
        if flags["bk"]:
            bkT_sb = consts.tile([P, NST], F32)
            nc.sync.dma_start(bkT_sb, bk2T.rearrange("(t p) o -> p (t o)", p=P))
        if flags["ff1b"]:
            f1bT_sb = consts.tile([P, NFT], F32)
            nc.sync.dma_start(f1bT_sb, ff1_bT.rearrange("(t p) o -> p (t o)", p=P))
        if flags["ff2b"]:
            f2b_sb = consts.tile([1, D], BF16)
            nc.sync.dma_start(f2b_sb, ff2_b)
        if flags["ln1g"]:
            ln1g_sb = consts.tile([P, D], F32, tag="ln1g")
            nc.sync.dma_start(ln1g_sb, bcast_row(ln1_g))
        if flags["ln1b"]:
            ln1b_sb = consts.tile([P, D], F32, tag="ln1b")
            nc.sync.dma_start(ln1b_sb, bcast_row(ln1_b))
        if flags["ln2g"]:
            ln2g_sb = consts.tile([P, D], F32, tag="ln2g")
            nc.sync.dma_start(ln2g_sb, bcast_row(ln2_g))
        if flags["ln2b"]:
            ln2b_sb = consts.tile([P, D], F32, tag="ln2b")
            nc.sync.dma_start(ln2b_sb, bcast_row(ln2_b))

        # transposed, normalized attention output [cin, q] (bf16)
        concatT = persist.tile([P, NKT, SQ], BF16, tag="concatT", name="concatT")
        # w0 lives in the top scope so its DMA can issue during the attention
        # phase; w1 streams with a prefetch ring that starts during W0
        w0_sb = persist.tile([P, NKT, D], BF16, tag="w0t")
        w1pool = top.enter_context(tc.tile_pool(name="w1p", bufs=6))
        w1_tiles = {}

        def w1_fetch(ft):
            w1 = w1pool.tile([P, NKT, P], BF16, tag="w1t", name=f"w1_{ft}")
            nc.sync.dma_start(flat(w1), f1_t[ft])
            w1_tiles[ft] = w1

        # ---------------- attention: single pass, 16 heads ----------------
        attn_scope = top.enter_context(ExitStack())
        hpool = attn_scope.enter_context(tc.tile_pool(name="hs", bufs=1))
        # q/k per-head tiles, zero-padded to K=128 partitions
        qt_pad = hpool.tile([P, H, SQ], BF16, tag="qt")
        kt_pad = hpool.tile([P, H, S], BF16, tag="kt")
        v_sb = hpool.tile([P, NST, H, DK + 1], BF16, tag="v")

        with ExitStack() as ph:
            xpool = ph.enter_context(tc.tile_pool(name="xs", bufs=1))
            vscope = ExitStack()
            vxpool = vscope.enter_context(tc.tile_pool(name="vx", bufs=1))

            # kick off all attention DMAs before any on-chip init so the
            # first V matmul can start as early as possible
            wv_sb = vxpool.tile([P, 2, NKT, 512], BF16, tag="wv")
            wvf = wv_sb.rearrange("p h a b -> p (h a b)")
            nc.sync.dma_start(wvf[:, 0:4 * 512], wv_t[0][:, 0:4 * 512])
            nc.sync.dma_start(wvf[:, 4 * 512:NKT * 512], wv_t[0][:, 4 * 512:])
            xv_ts = []
            for km in range(NST):
                xv = vxpool.tile([P, NKT, P], BF16, tag=f"xv{km}",
                                 name=f"xv{km}")
                nc.sync.dma_start(flat(xv), xv_t[km])
                xv_ts.append(xv)
            nc.sync.dma_start(wv_sb[:, 1].rearrange("p a b -> p (a b)"), wv_t[1])
            xq_sb = xpool.tile([P, NKT, SQ], BF16, tag="xq")
            nc.sync.dma_start(flat(xq_sb), xq_t)
            xk_sb = xpool.tile([P, NKT, S], BF16, tag="xk")
            nc.sync.dma_start(flat(xk_sb)[:, 0:NKT * 512], xk_t[:, 0:NKT * 512])
            nc.sync.dma_start(flat(xk_sb)[:, NKT * 512:], xk_t[:, NKT * 512:])

            # on-chip constant init on the (otherwise idle) GpSimd engine
            make_identity(nc, ident)
            nc.gpsimd.memset(eps_t, EPS)
            nc.gpsimd.memset(qt_pad[64:128, :, :], 0.0)
            nc.gpsimd.memset(kt_pad[64:128, :, :], 0.0)
            nc.vector.memset(v_sb[:, :, :, DK:DK + 1], 1.0)

            pp = ph.enter_context(tc.tile_pool(name="pp", bufs=2, space="PSUM"))
            ps = ph.enter_context(tc.tile_pool(name="ps", bufs=2, space="PSUM"))
            pa = ph.enter_context(tc.tile_pool(name="pa", bufs=2, space="PSUM"))

            # V projection first: all 16 heads, hdk-half outer so compute can
            # begin as soon as the first wv half lands
            for hf in range(2):
                for km in range(NST):
                    pv = pp.tile([P, 512], F32, tag="pp", name="pv")
                    for kt in range(NKT):
                        nc.tensor.matmul(pv, xv_ts[km][:, kt, :],
                                         wv_sb[:, hf, kt, :],
                                         start=(kt == 0),
                                         stop=(kt == NKT - 1 and not flags["bv"]))
                    if flags["bv"]:
                        nc.tensor.matmul(pv, ones_row,
                                         bv2_sb[:, hf * 512:hf * 512 + 512],
                                         start=False, stop=True)
                    nc.vector.tensor_copy(
                        v_sb[:, km, hf * 8:hf * 8 + 8, 0:DK],
                        pv.rearrange("p (s c) -> p s c", c=DK))
            # wv / xv space is dead from here on -- release it for the
            # eT / recip pools
            vscope.close()
            wpool = ph.enter_context(tc.tile_pool(name="w", bufs=5))
            epool = ph.enter_context(tc.tile_pool(name="e", bufs=3))
            rpool = ph.enter_context(tc.tile_pool(name="r", bufs=2))

            # q/k weight tiles stream ahead of the projection loop (2-deep
            # prefetch); w0 queues behind them -- it is not needed until the
            # attention phase ends
            qk_tiles = {}

            def qk_fetch(mt):
                wq_mt = wpool.tile([P, NKT, P], BF16, tag="wq_mt",
                                   name=f"wq{mt}")
                nc.sync.dma_start(flat(wq_mt), wq_t[mt])
                wk_mt = wpool.tile([P, NKT, P], BF16, tag="wk_mt",
                                   name=f"wk{mt}")
                nc.sync.dma_start(flat(wk_mt), wk_t[mt])
                qk_tiles[mt] = (wq_mt, wk_mt)

            qk_fetch(0)
            qk_fetch(1)
            nc.sync.dma_start(flat(w0_sb), w0_t)

            def scores_head(h):
                """8 score matmuls + exp into a fresh eT tile; returns eT."""
                eT = epool.tile([P, NST, SQ], BF16, tag="eT", name=f"eT{h}")
                for g in range(4):          # pairs of key tiles
                    psc = ps.tile([P, 2, 512], F32, tag="psc", name="psc")
                    for j in range(2):
                        st = 2 * g + j
                        nc.tensor.matmul(
                            psc[:, j, :],
                            kt_pad[:, h, st * P:(st + 1) * P],
                            qt_pad[:, h, :], start=True, stop=True)
                    if flags["mask"]:
                        for j in range(2):
                            st = 2 * g + j
                            nc.scalar.activation(
                                eT[:, st, :], psc[:, j, :], AF.Exp,
                                bias=mrow[:, st:st + 1])
                    else:
                        nc.scalar.activation(
                            eT[:, 2 * g:2 * g + 2, :], psc, AF.Exp)
                return eT

            def attnv_mm(h, eT):
                """aT = V'.T @ eT (row 64 = softmax denominator).  The
                softmax normalization is fused into the PSUM eviction: a
                per-head reciprocal of the denominator row is broadcast
                across partitions on GpSimd, then one tensor_tensor multiply
                evicts pat into concatT already normalized."""
                pat = pa.tile([DK + 1, SQ], F32, tag="pat", name="pat")
                for st in range(NST):
                    nc.tensor.matmul(pat, v_sb[:, st, h, :], eT[:, st, :],
                                     start=(st == 0), stop=(st == NST - 1))
                rjf = rpool.tile([1, SQ], F32, tag="rjf", name=f"rjf{h}",
                                 bufs=2)
                nc.vector.tensor_copy(rjf, pat[DK:DK + 1, :])
                nc.vector.reciprocal_approx_fast(rjf, rjf)
                rj = rpool.tile([1, SQ], BF16, tag="rj", name=f"rj{h}",
                                bufs=2)
                nc.vector.tensor_copy(rj, rjf)
                rb = rpool.tile([P, SQ], BF16, tag="rb", name=f"rb{h}",
                                bufs=3)
                nc.gpsimd.partition_broadcast(rb, rj, channels=P)
                po = (h % 2) * 64
                nc.vector.tensor_tensor(
                    out=concatT[po:po + 64, h // 2, :], in0=pat[0:DK, :],
                    in1=rb[po:po + 64, :], op=ALU.mult)

            # software pipeline: Q/K proj of tile mt, scores(h), attnV
            # matmuls of h-2.
            pending = []
            for mt in range(NKT):
                if mt + 2 < NKT:
                    qk_fetch(mt + 2)
                wq_mt, wk_mt = qk_tiles.pop(mt)
                pq = pp.tile([P, 512], F32, tag="pp", name="pq")
                for kt in range(NKT):
                    nc.tensor.matmul(pq, wq_mt[:, kt, :], xq_sb[:, kt, :],
                                     start=(kt == 0), stop=(kt == NKT - 1))
                for sub in range(2):
                    psl = slice(sub * 64, sub * 64 + 64)
                    # Q evictions ride the scalar engine to keep the DVE
                    # FIFO clear for the K / attnV eviction chain
                    nc.scalar.activation(
                        qt_pad[0:64, 2 * mt + sub, :], pq[psl, :],
                        AF.Identity,
                        bias=bqT_sb[psl, mt:mt + 1] if flags["bq"] else 0.0)

                for half in range(2):
                    fsl = slice(half * 512, half * 512 + 512)
                    pk = pp.tile([P, 512], F32, tag="pp", name="pk")
                    for kt in range(NKT):
                        nc.tensor.matmul(pk, wk_mt[:, kt, :],
                                         xk_sb[:, kt, fsl],
                                         start=(kt == 0), stop=(kt == NKT - 1))
                    for sub in range(2):
                        psl = slice(sub * 64, sub * 64 + 64)
                        if flags["bk"]:
                            nc.scalar.activation(
                                kt_pad[0:64, 2 * mt + sub, fsl], pk[psl, :],
                                AF.Identity, bias=bkT_sb[psl, mt:mt + 1])
                        else:
                            nc.vector.tensor_copy(
                                kt_pad[0:64, 2 * mt + sub, fsl], pk[psl, :])

                for h in (2 * mt, 2 * mt + 1):
                    eT = scores_head(h)
                    if len(pending) == 2:
                        hh, eTT = pending.pop(0)
                        attnv_mm(hh, eTT)
                    pending.append((h, eT))
            for hh, eTT in pending:
                attnv_mm(hh, eTT)

        # ---------------- mha out + residual + LN1 + attnw15 --------------
        sub1 = [persist.tile([P, D], F32, tag=f"sub1_{qt}", name=f"sub1_{qt}")
                for qt in range(NQT)]
        sub1T = persist.tile([P, NKT, SQ], BF16, tag="sub1T")
        with ExitStack() as ph:
            xpool = ph.enter_context(tc.tile_pool(name="xr", bufs=2))
            apool = ph.enter_context(tc.tile_pool(name="a15", bufs=2))
            lnpool = ph.enter_context(tc.tile_pool(name="ln1pool", bufs=4))
            po = ph.enter_context(tc.tile_pool(name="po", bufs=2, space="PSUM"))
            p15 = ph.enter_context(tc.tile_pool(name="p15", bufs=2, space="PSUM"))
            pt = ph.enter_context(tc.tile_pool(name="ptr", bufs=2, space="PSUM"))

            # start the FF1 weight stream while W0 runs
            for ft in range(6):
                w1_fetch(ft)

            def w0_block(qt):
                # kt outer / half inner: both halves stream from the same
                # stationary concatT tile, halving the weight switches
                pmo = po.tile([P, 2, 512], F32, tag="pmo", name="pmo")
                for kt in range(NKT):
                    for half in range(2):
                        fsl = slice(half * 512, half * 512 + 512)
                        nc.tensor.matmul(pmo[:, half, :],
                                         concatT[:, kt, qt * P:(qt + 1) * P],
                                         w0_sb[:, kt, fsl],
                                         start=(kt == 0),
                                         stop=(kt == NKT - 1 and not flags["b0"]))
                if flags["b0"]:
                    for half in range(2):
                        fsl = slice(half * 512, half * 512 + 512)
                        nc.tensor.matmul(pmo[:, half, :], ones_row,
                                         b0_sb[:, fsl], start=False, stop=True)
                xq = xpool.tile([P, D], BF16, tag="xqr")
                nc.sync.dma_start(xq, x_q_r[qt * P:(qt + 1) * P, :])
                nc.vector.tensor_add(sub1[qt], flat(pmo), xq)
                _layernorm(nc, lnpool, sub1[qt], eps_t, ln1g_sb, ln1b_sb)

            def t_block(qt):
                # transpose sub1 (f32) into sub1T [c, q] (bf16 on evict)
                for ct in range(NKT):
                    ptt = pt.tile([P, P], F32, tag="ptt", name="ptt")
                    nc.tensor.transpose(
                        ptt, sub1[qt][:, ct * P:(ct + 1) * P], ident)
                    nc.vector.tensor_copy(
                        sub1T[:, ct, qt * P:(qt + 1) * P], ptt)

            # interleave so each transpose block's LN1 is ready when the PE
            # reaches it
            w0_block(0)
            for qt in range(1, NQT):
                w0_block(qt)
                t_block(qt - 1)
            t_block(NQT - 1)

            # head-15 attention weights: recompute scores q-major (f32 path).
            # De-prioritized PE/ACT filler; the row-sum + normalize runs on
            # GpSimd (otherwise idle here) so the DVE stays clear for LN1 and
            # the sub1T transpose evictions.
            with tc.high_priority(offset=-180):
                for qt in range(NQT):
                    a15 = apool.tile([P, S], F32, tag="a15")
                    a15n = apool.tile([P, S], F32, tag="a15n", name="a15n")
                    for half in range(2):
                        fsl = slice(half * 512, half * 512 + 512)
                        pw = p15.tile([P, 512], F32, tag="p15", name="pw")
                        nc.tensor.matmul(
                            pw, qt_pad[:, H - 1, qt * P:(qt + 1) * P],
                            kt_pad[:, H - 1, fsl],
                            start=True, stop=not flags["mask"])
                        if flags["mask"]:
                            nc.tensor.matmul(pw, ones_row, mrow_r[:, fsl],
                                             start=False, stop=True)
                        nc.scalar.activation(a15[:, fsl], pw, AF.Exp)
                    den = apool.tile([P, 1], F32, tag="den15", name="den15")
                    nc.vector.tensor_reduce(den, a15, mybir.AxisListType.X,
                                            ALU.add)
                    nc.gpsimd.normalize_recip(a15n, a15, den)
                    nc.sync.dma_start(attnw15[qt * P:(qt + 1) * P, :], a15n)

        attn_scope.close()

        # ---------------- FFN + residual + LN2 ----------------
        with ExitStack() as ph:
            mpool = ph.enter_context(tc.tile_pool(name="f", bufs=1))
            w2pool = ph.enter_context(tc.tile_pool(name="w2p", bufs=1))
            lnpool = ph.enter_context(tc.tile_pool(name="ln2pool", bufs=4))
            pf = ph.enter_context(tc.tile_pool(name="pf", bufs=3, space="PSUM"))
            pg = ph.enter_context(tc.tile_pool(name="pg", bufs=2, space="PSUM"))

            w2_sb = w2pool.tile([P, NFT, D], BF16, tag="w2t")

            hT = mpool.tile([P, NFT, SQ], BF16, tag="hT")
            for ft in range(NFT):
                if ft + 6 < NFT:
                    w1_fetch(ft + 6)
                if ft == 16:
                    # FF2 weights: issued midway through FF1 so they queue
                    # behind the remaining FF1 stream but land before FF2
                    for c in range(8):
                        nc.sync.dma_start(
                            w2_sb[:, 4 * c:4 * c + 4, :],
                            f2_t.rearrange("p (a b) -> p a b", b=D)
                            [:, 4 * c:4 * c + 4, :])
                w1 = w1_tiles.pop(ft)
                pff = pf.tile([P, SQ], F32, tag="pff", name="pff")
                for kt in range(NKT):
                    nc.tensor.matmul(pff, w1[:, kt, :], sub1T[:, kt, :],
                                     start=(kt == 0), stop=(kt == NKT - 1))
                if flags["ff1b"]:
                    nc.vector.tensor_scalar(
                        out=hT[:, ft, :], in0=pff,
                        scalar1=f1bT_sb[:, ft:ft + 1], scalar2=0.0,
                        op0=ALU.add, op1=ALU.max)
                else:
                    nc.vector.tensor_scalar_max(
                        out=hT[:, ft, :], in0=pff, scalar1=0.0)

            for qt in range(NQT):
                # ft outer / half inner: both halves stream from the same
                # stationary hT tile, halving the weight switches
                pfn = pg.tile([P, 2, 512], F32, tag="pfn", name="pfn")
                for ft in range(NFT):
                    for half in range(2):
                        fsl = slice(half * 512, half * 512 + 512)
                        nc.tensor.matmul(
                            pfn[:, half, :],
                            hT[:, ft, qt * P:(qt + 1) * P],
                            w2_sb[:, ft, fsl],
                            start=(ft == 0),
                            stop=(ft == NFT - 1 and not flags["ff2b"]))
                if flags["ff2b"]:
                    for half in range(2):
                        fsl = slice(half * 512, half * 512 + 512)
                        nc.tensor.matmul(pfn[:, half, :], ones_row,
                                         f2b_sb[:, fsl], start=False, stop=True)
                nc.vector.tensor_add(sub1[qt], flat(pfn), sub1[qt])
                _layernorm(nc, lnpool, sub1[qt], eps_t, ln2g_sb, ln2b_sb,
                           out_dma=out_r[qt * P:(qt + 1) * P, :])

    nc.compile()
    return nc


_CACHE = {}


def _get_program(flags):
    key = tuple(sorted(flags.items()))
    if key not in _CACHE:
        _CACHE[key] = _build(flags)
    return _CACHE[key]


def make_flags(mask, bq, bk, bv, b0, ff1_b, ff2_b, ln1_g, ln1_b, ln2_g, ln2_b):
    return {
        "mask": bool(np.any(mask)), "bq": bool(np.any(bq)),
        "bk": bool(np.any(bk)), "bv": bool(np.any(bv)),
        "b0": bool(np.any(b0)), "ff1b": bool(np.any(ff1_b)),
        "ff2b": bool(np.any(ff2_b)),
        "ln1g": bool(np.any(ln1_g != 1.0)), "ln1b": bool(np.any(ln1_b)),
        "ln2g": bool(np.any(ln2_g != 1.0)), "ln2b": bool(np.any(ln2_b)),
    }


def make_in_maps(x_v, x_k, x_q, mask, wq, bq, wk, bk, wv, bv, w0, b0,
                 ln1_g, ln1_b, ff1_w, ff1_b, ff2_w, ff2_b, ln2_g, ln2_b):
    import ml_dtypes
    f32 = np.float32
    bf16 = ml_dtypes.bfloat16
    c = np.ascontiguousarray

    def cb(a):
        return c(np.asarray(a, f32).astype(bf16))

    wq2 = np.transpose(np.asarray(wq, f32), (1, 0, 2)).reshape(D, H * DK) / 8.0
    wk2 = np.transpose(np.asarray(wk, f32), (1, 0, 2)).reshape(D, H * DK)
    wv2 = np.transpose(np.asarray(wv, f32), (1, 0, 2)).reshape(D, H * DK)
    w0a = np.asarray(w0, f32)
    f1a = np.asarray(ff1_w, f32)
    f2a = np.asarray(ff2_w, f32)
    shared = {
        # [mt, p, kt*128+c] = wq2[kt*128+p, mt*128+c]
        "wq_t": cb(wq2.reshape(NKT, P, NKT, P).transpose(2, 1, 0, 3)
                   .reshape(NKT, P, NKT * P)),
        "wk_t": cb(wk2.reshape(NKT, P, NKT, P).transpose(2, 1, 0, 3)
                   .reshape(NKT, P, NKT * P)),
        # [hf, p, kt*512+c] = wv2[kt*128+p, hf*512+c]
        "wv_t": cb(wv2.reshape(NKT, P, 2, 512).transpose(2, 1, 0, 3)
                   .reshape(2, P, NKT * 512)),
        "bq2T": c(np.asarray(bq, f32).reshape(H * DK, 1) / 8.0),
        "bk2T": c(np.asarray(bk, f32).reshape(H * DK, 1)),
        "bv2": cb(np.asarray(bv, f32).reshape(1, H * DK)),
        # [p, kt*1024+c] = w0[kt*128+p, c]
        "w0_t": cb(w0a.reshape(NKT, P, D).transpose(1, 0, 2).reshape(P, NKT * D)),
        "b0": cb(np.asarray(b0, f32).reshape(1, D)),
        # [ft, p, kt*128+c] = ff1_w[kt*128+p, ft*128+c]
        "f1_t": cb(f1a.reshape(NKT, P, NFT, P).transpose(2, 1, 0, 3)
                   .reshape(NFT, P, NKT * P)),
        "ff1_bT": c(np.asarray(ff1_b, f32).reshape(DFF, 1)),
        # [p, ft*1024+d] = ff2_w[ft*128+p, d]
        "f2_t": cb(f2a.reshape(NFT, P, D).transpose(1, 0, 2).reshape(P, NFT * D)),
        "ff2_b": cb(np.asarray(ff2_b, f32).reshape(1, D)),
        "ln1_g": c(np.asarray(ln1_g, f32).reshape(1, D)),
        "ln1_b": c(np.asarray(ln1_b, f32).reshape(1, D)),
        "ln2_g": c(np.asarray(ln2_g, f32).reshape(1, D)),
        "ln2_b": c(np.asarray(ln2_b, f32).reshape(1, D)),
    }
    in_maps = []
    for core in range(8):
        b, half = core // 2, core % 2
        rows = slice(half * SQ, (half + 1) * SQ)
        xqb = np.asarray(x_q[b], f32)[rows]            # [512, 1024]
        xkb = np.asarray(x_k[b], f32)                  # [1024, 1024]
        xvb = np.asarray(x_v[b], f32)
        m = dict(shared)
        # [p, kt*512+c] = x_q[b, half*512+c, kt*128+p]
        m["xq_t"] = cb(xqb.reshape(SQ, NKT, P).transpose(2, 1, 0)
                       .reshape(P, NKT * SQ))
        m["x_q_r"] = cb(xqb)
        # [p, kt*1024+c] = x_k[b, c, kt*128+p]
        m["xk_t"] = cb(xkb.reshape(S, NKT, P).transpose(2, 1, 0)
                       .reshape(P, NKT * S))
        # [km, p, kt*128+c] = x_v[b, km*128+c, kt*128+p]
        m["xv_t"] = cb(xvb.reshape(NST, P, NKT, P).transpose(0, 3, 2, 1)
                       .reshape(NST, P, NKT * P))
        m["maskT"] = c(np.asarray(mask[b], f32).reshape(1, S).T)
        in_maps.append(m)
    return in_maps


def kernel(x_v, x_k, x_q, mask, wq, bq, wk, bk, wv, bv, w0, b0,
           ln1_g, ln1_b, ff1_w, ff1_b, ff2_w, ff2_b, ln2_g, ln2_b,
           _trace=False):
    from concourse import bass_utils

    flags = make_flags(mask, bq, bk, bv, b0, ff1_b, ff2_b,
                       ln1_g, ln1_b, ln2_g, ln2_b)
    nc = _get_program(flags)
    in_maps = make_in_maps(x_v, x_k, x_q, mask, wq, bq, wk, bk, wv, bv,
                           w0, b0, ln1_g, ln1_b, ff1_w, ff1_b,
                           ff2_w, ff2_b, ln2_g, ln2_b)
    res = bass_utils.run_bass_kernel_spmd(
        nc, in_maps, core_ids=list(range(8)), trace=_trace)

    out = np.empty((B, S, D), np.float32)
    attn = np.empty((B, S, S), np.float32)
    for core in range(8):
        b, half = core // 2, core % 2
        rows = slice(half * SQ, (half + 1) * SQ)
        out[b, rows] = res.results[core]["out_r"]
        attn[b, rows] = res.results[core]["attnw15"]
    if _trace:
        kernel.last_exec_time_ns = res.exec_time_ns
        kernel.last_trace = (res.instructions_and_trace or (None, None))[1]
        kernel.last_insts = (res.instructions_and_trace or (None, None))[0]
    return out, attn

